# revision 18
# baseline (speedup 1.0000x reference)
"""Trainium2 Bass kernel for nn_RecQNetwork (GRU-based recurrent Q network).

Data-parallel over the batch axis: 8 NeuronCores, each runs the full T=512
scan for its shard of B*A = 1024/8 = 128 (b, a) sequences.

Orientation on device: state and gates are kept TRANSPOSED as [H, BA] so the
recurrent matmuls (stationary gate weights, moving state) emit the new state
directly in matmul orientation, with no per-step transposes.

Outputs are written in device-friendly transposed layouts and fixed up on the
host (q as [T/4, NA, 4*BA]; h_final as [H, BA]).
"""

import os
import sys

for _p in ("/opt/trn_rl_repo", "/root/.axon_site/_ro/trn_rl_repo"):
    if os.path.isdir(_p) and _p not in sys.path:
        sys.path.insert(0, _p)

from contextlib import ExitStack

import ml_dtypes
import numpy as np

import concourse.bass as bass
import concourse.tile as tile
from concourse import bacc, mybir
from concourse.bass_utils import run_bass_kernel_spmd

BF16 = mybir.dt.bfloat16
F32 = mybir.dt.float32
F32R = mybir.dt.float32r
AF = mybir.ActivationFunctionType
OP = mybir.AluOpType

T, B, A, O, H, NA = 512, 128, 8, 128, 128, 32
NCORES = 8
BSH = B // NCORES      # 16 batch rows per core
BA = BSH * A           # 128 sequences per core
G3 = 3 * H


def build_program(t_steps=T, has_bias=False, debug_taps=0):
    """Build the SPMD per-core program. has_bias adds the (normally zero)
    bias terms exactly. debug_taps>0 dumps per-step intermediates for the
    first debug_taps steps."""
    nc = bacc.Bacc("TRN2", debug=False, num_devices=NCORES)

    # ---- DRAM I/O (per core) ----
    obsT = nc.dram_tensor("obsT", [O, t_steps, BA], BF16, kind="ExternalInput").ap()
    kmask = nc.dram_tensor("kmask", [t_steps, BA], BF16, kind="ExternalInput").ap()
    hT0 = nc.dram_tensor("hT0", [H, BA], F32, kind="ExternalInput").ap()
    w_pre = nc.dram_tensor("w_pre", [O, H], BF16, kind="ExternalInput").ap()
    w_i = nc.dram_tensor("w_i", [H, G3], BF16, kind="ExternalInput").ap()
    w_h = nc.dram_tensor("w_h", [H, G3], BF16, kind="ExternalInput").ap()
    w_post = nc.dram_tensor("w_post", [H, H], F32R, kind="ExternalInput").ap()
    w_out = nc.dram_tensor("w_out", [H, NA], F32R, kind="ExternalInput").ap()
    if has_bias:
        b_pre = nc.dram_tensor("b_pre", [H, 1], F32, kind="ExternalInput").ap()
        b_i = nc.dram_tensor("b_i", [G3, 1], F32, kind="ExternalInput").ap()
        b_hn = nc.dram_tensor("b_hn", [H, 1], F32, kind="ExternalInput").ap()
        b_post = nc.dram_tensor("b_post", [H, 1], F32, kind="ExternalInput").ap()
        b_out = nc.dram_tensor("b_out", [NA, 1], F32, kind="ExternalInput").ap()

    n_q = t_steps // 4
    q_out = nc.dram_tensor("q_out", [n_q, NA, 4 * BA], F32, kind="ExternalOutput").ap()
    hT_fin = nc.dram_tensor("hT_fin", [H, BA], F32R, kind="ExternalOutput").ap()
    if debug_taps:
        dbg_rz = nc.dram_tensor("dbg_rz", [debug_taps, H, 2 * BA], F32,
                                kind="ExternalOutput").ap()
        dbg_n = nc.dram_tensor("dbg_n", [debug_taps, H, BA], F32,
                               kind="ExternalOutput").ap()
        dbg_h = nc.dram_tensor("dbg_h", [debug_taps, H, BA], F32R,
                               kind="ExternalOutput").ap()
        dbg_hm = nc.dram_tensor("dbg_hm", [debug_taps, H, BA], F32,
                                kind="ExternalOutput").ap()
        dbg_emb = nc.dram_tensor("dbg_emb", [debug_taps, H, BA], F32,
                                 kind="ExternalOutput").ap()

    KCH = 16  # k-mask staging chunk (steps)

    with tile.TileContext(nc) as tc, ExitStack() as ctx:
        P = lambda **kw: ctx.enter_context(tc.tile_pool(**kw))
        singles = P(name="singles", bufs=1)
        obs_pool = P(name="obs", bufs=6)
        emb_pool = P(name="emb", bufs=4)
        rzsb_pool = P(name="rzsb", bufs=3)
        sc_pool = P(name="scan", bufs=3)
        hm_pool = P(name="hm", bufs=3)
        ys_pool = P(name="ys", bufs=3)
        post_pool = P(name="post", bufs=2)
        kst_pool = P(name="kst", bufs=2)
        kbc_pool = P(name="kbc", bufs=2)
        ps_rz = P(name="ps_rz", bufs=2, space="PSUM")
        ps_n = P(name="ps_n", bufs=2, space="PSUM")
        ps_emb = P(name="ps_emb", bufs=1, space="PSUM")
        ps_head = P(name="ps_head", bufs=1, space="PSUM")

        # ---- load constants ----
        w_pre_sb = singles.tile([O, H], BF16)
        nc.sync.dma_start(out=w_pre_sb, in_=w_pre)
        w_i_sb = singles.tile([H, G3], BF16)
        nc.sync.dma_start(out=w_i_sb, in_=w_i)
        w_h_sb = singles.tile([H, G3], BF16)
        nc.sync.dma_start(out=w_h_sb, in_=w_h)
        w_post_sb = singles.tile([H, H], F32R)
        nc.sync.dma_start(out=w_post_sb, in_=w_post)
        w_out_sb = singles.tile([H, NA], F32R)
        nc.sync.dma_start(out=w_out_sb, in_=w_out)
        if has_bias:
            b_pre_sb = singles.tile([H, 1], F32)
            nc.sync.dma_start(out=b_pre_sb, in_=b_pre)
            b_i_sb = singles.tile([G3, 1], F32)
            nc.sync.dma_start(out=b_i_sb, in_=b_i)
            b_hn_sb = singles.tile([H, 1], F32)
            nc.sync.dma_start(out=b_hn_sb, in_=b_hn)
            b_post_sb = singles.tile([H, 1], F32)
            nc.sync.dma_start(out=b_post_sb, in_=b_post)
            b_out_sb = singles.tile([NA, 1], F32)
            nc.sync.dma_start(out=b_out_sb, in_=b_out)

        h0_sb = singles.tile([H, BA], F32)
        nc.sync.dma_start(out=h0_sb, in_=hT0)

        # staged k-mask broadcast tiles, one chunk of KCH steps at a time
        def stage_k(chunk):
            t0 = chunk * KCH
            nst = min(KCH, t_steps - t0)
            kbc = kbc_pool.tile([128, KCH * BA], BF16)
            src = (
                kmask[t0 : t0 + nst, :]
                .rearrange("t b -> (t b)")
                .unsqueeze(0)
                .partition_broadcast(128)
            )
            nc.sync.dma_start(out=kbc[:, : nst * BA], in_=src)
            return kbc

        kbc_cur = stage_k(0)

        def kb_t(t):
            return kbc_cur[:, (t % KCH) * BA : (t % KCH + 1) * BA]

        # initial masked state (bf16) = hT0 * K_0
        hm = hm_pool.tile([H, BA], BF16, tag="hm")
        nc.vector.tensor_tensor(hm, h0_sb, kb_t(0), OP.mult)

        emb_bf = None
        ys4 = None
        rz_bank = n_bank = None
        rz_v = n_v = None

        for t in range(t_steps):
            s4 = t % 4
            s2 = t % 2

            if t % KCH == 0 and t > 0:
                kbc_cur = stage_k(t // KCH)

            # ---- phase A: pre-torso (every 4 steps) ----
            if s4 == 0:
                n4 = min(4, t_steps - t)
                obst = obs_pool.tile([O, 4, BA], BF16, tag="obst")
                nc.sync.dma_start(out=obst[:, :n4, :], in_=obsT[:, t : t + n4, :])
                pe = ps_emb.tile([H, 4 * BA], F32, tag="pe")
                nc.tensor.matmul(
                    pe[:, : n4 * BA],
                    lhsT=w_pre_sb,
                    rhs=obst[:, :n4, :].rearrange("p a b -> p (a b)"),
                    start=True,
                    stop=True,
                )
                emb_bf = emb_pool.tile([H, 4 * BA], BF16, tag="emb")
                if has_bias:
                    nc.vector.tensor_scalar(
                        emb_bf[:, : n4 * BA], pe[:, : n4 * BA],
                        b_pre_sb, 0.0, OP.add, OP.max,
                    )
                else:
                    nc.vector.tensor_scalar_max(emb_bf[:, : n4 * BA], pe[:, : n4 * BA], 0.0)

            # ---- phase A: gi matmuls (every 2 steps) ----
            if s2 == 0:
                n2 = min(2, t_steps - t)
                nb = n2 * BA
                eoff = s4 * BA
                erhs = emb_bf[:, eoff : eoff + nb]
                # rz_bank spans 2 PSUM banks: bank0=[r_t0 r_t1 pad],
                # bank1=[z_t0 z_t1 pad].  PSUM accumulation groups are
                # per-bank, so gi_r/gi_z are issued per step inside the scan
                # (open group -> gh accumulate closes it -> sigmoid reads).
                rz_bank = ps_rz.tile([128, 1024], F32, tag="rz")
                # n_bank free layout:  [gin_t0 gin_t1 hn_t0 hn_t1]
                n_bank = ps_n.tile([128, 512], F32, tag="nb")
                nc.tensor.matmul(n_bank[:, :nb], lhsT=w_i_sb[:, 2 * H :], rhs=erhs,
                                 start=True, stop=True)
                rz_v = rz_bank.rearrange("p (g x) -> p g x", g=2)
                n_v = n_bank.rearrange("p (c s b) -> p c s b", c=2, s=2)

            # ---- scan step ----
            r_sl = rz_v[:, 0, s2 * BA : (s2 + 1) * BA]
            z_sl = rz_v[:, 1, s2 * BA : (s2 + 1) * BA]
            gin_sl = n_v[:, 0, s2, :]
            hn_sl = n_v[:, 1, s2, :]
            e_sl = emb_bf[:, s4 * BA : (s4 + 1) * BA]

            # per-step gi (opens the bank group), gh accumulates and closes
            nc.tensor.matmul(r_sl, lhsT=w_i_sb[:, 0:H], rhs=e_sl, start=True,
                             stop=False)
            nc.tensor.matmul(r_sl, lhsT=w_h_sb[:, 0:H], rhs=hm, start=False,
                             stop=True)
            nc.tensor.matmul(hn_sl, lhsT=w_h_sb[:, 2 * H :], rhs=hm, start=True, stop=True)
            nc.tensor.matmul(z_sl, lhsT=w_i_sb[:, H : 2 * H], rhs=e_sl, start=True,
                             stop=False)
            nc.tensor.matmul(z_sl, lhsT=w_h_sb[:, H : 2 * H], rhs=hm, start=False,
                             stop=True)

            # sigmoid over [r_t | z_t] (strided in PSUM, contiguous out)
            rz_sb = rzsb_pool.tile([128, 256], F32, tag="rzsb")
            if has_bias:
                nc.scalar.activation(rz_sb[:, 0:H], r_sl, AF.Sigmoid, bias=b_i_sb[0:H, :])
                nc.scalar.activation(rz_sb[:, H:], z_sl, AF.Sigmoid, bias=b_i_sb[H : 2 * H, :])
            else:
                nc.scalar.activation(rz_sb, rz_v[:, :, s2 * BA : (s2 + 1) * BA], AF.Sigmoid)
            rg = rz_sb[:, 0:H]
            zg = rz_sb[:, H:]

            # n_pre = gi_n (+ b_i_n) + rg * (h_n + b_hn)
            if has_bias:
                hn_b = sc_pool.tile([H, BA], F32, tag="hnb")
                nc.vector.tensor_scalar_add(hn_b, hn_sl, b_hn_sb)
                hn_in = hn_b
            else:
                hn_in = hn_sl
            m1 = sc_pool.tile([H, BA], F32, tag="m1")
            nc.vector.tensor_tensor(m1, rg, hn_in, OP.mult)
            npre = sc_pool.tile([H, BA], F32, tag="npre")
            nc.vector.tensor_tensor(npre, m1, gin_sl, OP.add)
            if has_bias:
                nc.vector.tensor_scalar_add(npre, npre, b_i_sb[2 * H :, :])
            n_sb = sc_pool.tile([H, BA], F32, tag="nsb")
            nc.scalar.activation(n_sb, npre, AF.Tanh)

            # zp = 1 - z (early);  pa = z * h_masked (early)
            zp = sc_pool.tile([H, BA], F32, tag="zp")
            nc.vector.tensor_scalar(zp, zg, -1.0, 1.0, OP.mult, OP.add)
            pa = sc_pool.tile([H, BA], F32, tag="pa")
            nc.vector.tensor_tensor(pa, zg, hm, OP.mult)

            # new_h (unmasked) = (1-z)*n + z*h  -> written into ys4 slice
            if s4 == 0:
                ys4 = ys_pool.tile([H, 4 * BA], F32R, tag="ys4")
            mb = sc_pool.tile([H, BA], F32, tag="mb")
            nc.vector.tensor_tensor(mb, zp, n_sb, OP.mult)
            hraw = ys4[:, s4 * BA : (s4 + 1) * BA]
            nc.vector.tensor_tensor(hraw, mb, pa, OP.add)

            if t < debug_taps:
                nc.sync.dma_start(out=dbg_rz[t], in_=rz_sb)
                nc.sync.dma_start(out=dbg_n[t], in_=n_sb)
                nc.sync.dma_start(out=dbg_h[t], in_=hraw)
                hm_f = sc_pool.tile([H, BA], F32, tag="dbghm")
                nc.vector.tensor_copy(hm_f, hm)
                nc.sync.dma_start(out=dbg_hm[t], in_=hm_f)
                emb_f = sc_pool.tile([H, BA], F32, tag="dbgemb")
                nc.vector.tensor_copy(emb_f, e_sl)
                nc.sync.dma_start(out=dbg_emb[t], in_=emb_f)

            if t + 1 < t_steps:
                # masked bf16 state for next step
                kb_next = (
                    kbc_cur[:, ((t + 1) % KCH) * BA : ((t + 1) % KCH + 1) * BA]
                    if (t + 1) % KCH != 0
                    else None
                )
                if kb_next is None:
                    kbc_cur = stage_k((t + 1) // KCH)
                    kb_next = kbc_cur[:, 0:BA]
                hm = hm_pool.tile([H, BA], BF16, tag="hm")
                nc.vector.tensor_tensor(hm, hraw, kb_next, OP.mult)
            else:
                nc.sync.dma_start(out=hT_fin, in_=hraw)

            # ---- post torso + q head (every 4 steps, after ys4 complete) ----
            if s4 == 3 or t == t_steps - 1:
                nb4 = (s4 + 1) * BA
                pp = ps_head.tile([H, 4 * BA], F32, tag="head")
                nc.tensor.matmul(
                    pp[:, :nb4],
                    lhsT=w_post_sb,
                    rhs=ys4[:, :nb4],
                    start=True,
                    stop=True,
                )
                post_sb = post_pool.tile([H, 4 * BA], F32R, tag="post")
                if has_bias:
                    nc.scalar.activation(post_sb[:, :nb4], pp[:, :nb4], AF.Relu,
                                         bias=b_post_sb)
                else:
                    nc.scalar.activation(post_sb[:, :nb4], pp[:, :nb4], AF.Relu)
                pq = ps_head.tile([NA, 4 * BA], F32, tag="head")
                nc.tensor.matmul(
                    pq[:, :nb4],
                    lhsT=w_out_sb,
                    rhs=post_sb[:, :nb4],
                    start=True,
                    stop=True,
                )
                q_sb = post_pool.tile([NA, 4 * BA], F32, tag="qsb")
                if has_bias:
                    nc.vector.tensor_scalar_add(q_sb[:, :nb4], pq[:, :nb4], b_out_sb)
                else:
                    nc.scalar.copy(q_sb[:, :nb4], pq[:, :nb4])
                nc.sync.dma_start(out=q_out[t // 4], in_=q_sb[:, :nb4])

    nc.compile()
    return nc


_PROG_CACHE = {}


def _get_program(t_steps, has_bias):
    key = (t_steps, has_bias)
    if key not in _PROG_CACHE:
        _PROG_CACHE[key] = build_program(t_steps, has_bias)
    return _PROG_CACHE[key]


def make_in_maps(hidden_state, obs, resets, W_pre, b_pre, Wi, bi, Wh, bhn,
                 W_post, b_post, W_out, b_out, t_steps=T):
    """Host-side prep: shard over B, transpose obs, cast weights."""
    has_bias = any(
        np.any(np.asarray(x) != 0) for x in (b_pre, bi, bhn, b_post, b_out)
    )
    bf = ml_dtypes.bfloat16
    w_pre_bf = np.ascontiguousarray(np.asarray(W_pre, np.float32).astype(bf))
    w_i_bf = np.ascontiguousarray(np.asarray(Wi, np.float32).astype(bf))
    w_h_bf = np.ascontiguousarray(np.asarray(Wh, np.float32).astype(bf))
    w_post_f = np.ascontiguousarray(np.asarray(W_post, np.float32))
    w_out_f = np.ascontiguousarray(np.asarray(W_out, np.float32))

    obs = np.asarray(obs, np.float32)
    resets = np.asarray(resets)
    hidden_state = np.asarray(hidden_state, np.float32)
    km_full = (~resets.astype(bool)).astype(np.float32).reshape(t_steps, B * A)

    in_maps = []
    for c in range(NCORES):
        b0 = c * BSH
        ob = obs[:, b0 : b0 + BSH].reshape(t_steps, BA, O)
        obT = np.ascontiguousarray(ob.transpose(2, 0, 1)).astype(bf)
        km = np.ascontiguousarray(km_full[:, b0 * A : (b0 + BSH) * A]).astype(bf)
        h0T = np.ascontiguousarray(
            hidden_state[b0 : b0 + BSH].reshape(BA, H).T
        )
        m = dict(obsT=obT, kmask=km, hT0=h0T, w_pre=w_pre_bf, w_i=w_i_bf,
                 w_h=w_h_bf, w_post=w_post_f, w_out=w_out_f)
        if has_bias:
            m.update(
                b_pre=np.asarray(b_pre, np.float32).reshape(H, 1),
                b_i=np.asarray(bi, np.float32).reshape(G3, 1),
                b_hn=np.asarray(bhn, np.float32).reshape(H, 1),
                b_post=np.asarray(b_post, np.float32).reshape(H, 1),
                b_out=np.asarray(b_out, np.float32).reshape(NA, 1),
            )
        in_maps.append(m)
    return in_maps, has_bias


def assemble_outputs(results, t_steps=T):
    """Gather per-core transposed outputs into full reference-shaped arrays."""
    h_fin = np.empty((B, A, H), np.float32)
    q = np.empty((t_steps, B, A, NA), np.float32)
    for c, res in enumerate(results):
        b0 = c * BSH
        h_fin[b0 : b0 + BSH] = res["hT_fin"].T.reshape(BSH, A, H)
        # q_out: [T/4, NA, 4*BA] -> [T/4, NA, 4, BA] -> [T/4, 4, BA, NA]
        qc = res["q_out"].reshape(t_steps // 4, NA, 4, BA)
        qc = qc.transpose(0, 2, 3, 1).reshape(t_steps, BSH, A, NA)
        q[:, b0 : b0 + BSH] = qc
    return h_fin, q


def kernel(hidden_state, obs, resets, W_pre, b_pre, Wi, bi, Wh, bhn,
           W_post, b_post, W_out, b_out):
    in_maps, has_bias = make_in_maps(
        hidden_state, obs, resets, W_pre, b_pre, Wi, bi, Wh, bhn,
        W_post, b_post, W_out, b_out,
    )
    nc = _get_program(T, has_bias)
    res = run_bass_kernel_spmd(nc, in_maps, list(range(NCORES)))
    h_fin, q = assemble_outputs(res.results)
    return h_fin, q
